# revision 1
# baseline (speedup 1.0000x reference)
"""Causal multi-head attention with RoPE on Trainium2, 8 NeuronCores.

Head-parallel sharding: 16 heads / 8 cores = 2 heads per core. Each core:
q/k/v projections for its 2 heads (128 of 1024 hidden dims), flash-style
causal attention in transposed-score layout (keys on partitions, softmax
denominator from an appended ones-column in V), row-parallel slice of the
output projection. Host sums the 8 partial (D,S) outputs.

v2 over baseline:
- softmax exp split between ScalarE (activation Exp) and VectorE (two custom
  DVE ops: degree-4 Horner poly for e^t on t = s/512, then ((x+b0)*a4)^64).
  Wq is pre-scaled by 1/(8*64) on host so scores arrive as t; the ACT path
  uses scale=64 to recover exp(s/8) identically.
- V^T produced directly from projection matmuls (out [keys,128]), no PE
  transposes.
- leaner epilogue: ACT does the l-row copies, single fused reciprocal.
- attention lags projections by one block so PE keeps dense work queued.

Self-contained: hardcodes B=1, S=4096, D=1024, H=16, hd=64.
"""

import sys

if "/opt/trn_rl_repo" not in sys.path:
    sys.path.insert(0, "/opt/trn_rl_repo")

import numpy as np

S = 4096
D = 1024
H = 16
HD = 64
NCORE = 8
P = 128
QB = 512          # query block width
NQB = S // QB     # 8
KC = 128          # key chunk
THETA = 10000.0

# exp split: chunk index c (global) goes to DVE when c % DVE_EXP_MOD == DVE_EXP_PHASE
DVE_EXP_MOD = 4
DVE_EXP_PHASE = 3

# custom exp constants (deg-4 minimax for e^t on [-0.4, 0.4], leading=1)
EB3 = 4.069521
EB2 = 12.099517
EB1 = 24.194042
EB0 = 24.195307
EA4 = 0.041330267
QSCALE = 0.125 / 64.0   # folded into Wq on host

_NC_CACHE = {}


def _register_custom_ops():
    from concourse.dve_spec import Spec, Src0, C0, C1, C2, lower
    from concourse.dve_uop import DveOpSpec
    from concourse import dve_ops as dvo

    def reg(name, spec):
        if name in dvo._SUB_OPCODE_FOR_NAME:
            return next(o for o in dvo.OPS if o.name == name)
        shas = {}
        for ver in ("v3",):
            uops = lower(spec, ver=ver)
            shas[ver] = DveOpSpec(name=name, opcode=0, uops=uops,
                                  rd1_en=False).sha(ver)
        op = dvo.DveOp(name, spec, subdim=False, uops_sha=shas)
        dvo.OPS.append(op)
        dvo.CUSTOM_DVE_SPECS[name] = spec
        dvo._SUB_OPCODE_FOR_NAME[name] = dvo._CUSTOM_DVE_ROW_BASE + len(dvo.OPS) - 1
        assert dvo._SUB_OPCODE_FOR_NAME[name] < 0x20
        return op

    f = np.float32
    poly = reg("EXP_POLY3M", Spec(
        body=(((Src0 + C0) * Src0 + C1) * Src0 + C2) * Src0,
        reference=lambda in0, in1, s0, s1, imm2: (
            (((in0.astype(np.float32) + f(s0)) * in0 + f(s1)) * in0 + f(imm2))
            * in0).astype(np.float32)))

    _m = (Src0 + C1) * C0
    _b = _m * _m
    for _ in range(5):
        _b = _b * _b

    def _refp(in0, in1, s0, s1, imm2):
        x = ((in0.astype(np.float32) + f(s1)) * f(s0)).astype(np.float32)
        for _ in range(6):
            x = (x * x).astype(np.float32)
        return x

    pw = reg("ADD_SCALE_POW64", Spec(body=_b, reference=_refp))
    return poly, pw


def _build_nc():
    import concourse.bacc as bacc
    import concourse.mybir as mybir
    from concourse.tile import TileContext
    from contextlib import ExitStack

    EXP_POLY3M, ADD_SCALE_POW64 = _register_custom_ops()

    F32 = mybir.dt.float32
    BF16 = mybir.dt.bfloat16
    EXP = mybir.ActivationFunctionType.Exp

    nc = bacc.Bacc("TRN2", target_bir_lowering=False)

    xT = nc.dram_tensor("xT", [D, S], BF16, kind="ExternalInput")
    wq = nc.dram_tensor("wq", [D, P], BF16, kind="ExternalInput")
    wk = nc.dram_tensor("wk", [D, P], BF16, kind="ExternalInput")
    wv = nc.dram_tensor("wv", [D, P], BF16, kind="ExternalInput")
    wo = nc.dram_tensor("wo", [P, D], BF16, kind="ExternalInput")
    cs = nc.dram_tensor("cs", [P, S], F32, kind="ExternalInput")
    sn = nc.dram_tensor("sn", [P, S], F32, kind="ExternalInput")
    cst = nc.dram_tensor("cst", [P, 2 * P], BF16, kind="ExternalInput")
    y = nc.dram_tensor("y", [D, S], F32, kind="ExternalOutput")

    xTr = xT.rearrange("(o p) s -> p o s", p=P)   # [128, 8, 4096]
    wqr = wq.rearrange("(o p) m -> p o m", p=P)   # [128, 8, 128]
    wkr = wk.rearrange("(o p) m -> p o m", p=P)
    wvr = wv.rearrange("(o p) m -> p o m", p=P)

    with TileContext(nc) as tc, ExitStack() as ctx:
        con = ctx.enter_context(tc.tile_pool(name="con", bufs=1))
        xp = ctx.enter_context(tc.tile_pool(name="xp", bufs=10))
        tp = ctx.enter_context(tc.tile_pool(name="tp", bufs=3))
        ptp = ctx.enter_context(tc.tile_pool(name="ptp", bufs=5))
        plp = ctx.enter_context(tc.tile_pool(name="plp", bufs=3))
        onp_ = ctx.enter_context(tc.tile_pool(name="onp", bufs=2))
        rlp = ctx.enter_context(tc.tile_pool(name="rlp", bufs=2))
        fsp = ctx.enter_context(tc.tile_pool(name="fsp", bufs=4))
        pp = ctx.enter_context(tc.tile_pool(name="pp", bufs=2, space="PSUM"))
        scp = ctx.enter_context(tc.tile_pool(name="scp", bufs=2, space="PSUM"))
        oap = ctx.enter_context(tc.tile_pool(name="oap", bufs=1, space="PSUM"))
        obp = ctx.enter_context(tc.tile_pool(name="obp", bufs=1, space="PSUM"))

        # ---- constants / weights ----
        wq_sb = con.tile([P, 8, P], BF16)
        nc.sync.dma_start(out=wq_sb, in_=wqr)
        wk_sb = con.tile([P, 8, P], BF16)
        nc.sync.dma_start(out=wk_sb, in_=wkr)
        wv_sb = con.tile([P, 8, P], BF16)
        nc.gpsimd.dma_start(out=wv_sb, in_=wvr)
        wo_sb = con.tile([P, D], BF16)
        nc.scalar.dma_start(out=wo_sb, in_=wo[:, :])
        cs_sb = con.tile([P, S], F32)
        sn_sb = con.tile([P, S], F32)
        nc.sync.dma_start(out=cs_sb[:, 0:QB], in_=cs[:, 0:QB])
        nc.sync.dma_start(out=sn_sb[:, 0:QB], in_=sn[:, 0:QB])
        nc.scalar.dma_start(out=cs_sb[:, QB:S], in_=cs[:, QB:S])
        nc.scalar.dma_start(out=sn_sb[:, QB:S], in_=sn[:, QB:S])
        cst_sb = con.tile([P, 2, P], BF16)
        nc.sync.dma_start(out=cst_sb, in_=cst.rearrange("p (t m) -> p t m", t=2))
        pswap_sb = cst_sb[:, 0, :]
        tri_sb = cst_sb[:, 1, :]

        # v in [key, hd] layout per head + ones column at index HD
        vna = [
            con.tile([P, S // KC, HD + 1], BF16, tag=f"vna{h}", name=f"vna{h}")
            for h in (0, 1)
        ]
        for h in (0, 1):
            nc.vector.memset(vna[h][:, :, HD : HD + 1], 1.0)

        qTr = con.tile([P, S], BF16, tag="qTr")
        kTr = con.tile([P, S], BF16, tag="kTr")

        # HAM warmup: dependency-free matmuls on scratch tiles keep the PE
        # busy (and its clock at 8/8) while the first DMAs land.
        wsc_w = con.tile([P, P], BF16, tag="wscw")
        wsc_x = con.tile([P, QB], BF16, tag="wscx")
        nc.vector.memset(wsc_w, 1.0)
        nc.vector.memset(wsc_x, 1.0)
        wups = oap.tile([P, QB], F32, tag="oA", name="warm")
        for _ in range(40):
            nc.tensor.matmul(wups, wsc_w, wsc_x, start=True, stop=True)

        def emit_proj(st):
            sl = slice(st * QB, (st + 1) * QB)
            xts = []
            for dk in range(8):
                xt = xp.tile([P, QB], BF16, tag="x")
                nc.sync.dma_start(out=xt, in_=xTr[:, dk, sl])
                xts.append(xt)

            accs = {}
            for key, wsb in (("q", wq_sb), ("k", wk_sb)):
                acc = pp.tile([P, QB], F32, tag="ps", name=f"acc{key}")
                for dk in range(8):
                    nc.tensor.matmul(acc, wsb[:, dk, :], xts[dk],
                                     start=(dk == 0), stop=(dk == 7))
                accs[key] = acc
            raws = {}
            for key in ("q", "k"):
                raw = tp.tile([P, QB], BF16, tag="raw", name=f"raw{key}")
                nc.vector.tensor_copy(out=raw, in_=accs[key])
                raws[key] = raw
            t1s = {}
            for key, csb in (("q", cs_sb), ("k", cs_sb)):
                t1 = tp.tile([P, QB], F32, tag=f"t1{key}", name=f"t1{key}")
                nc.vector.tensor_mul(out=t1, in0=accs[key], in1=csb[:, sl])
                t1s[key] = t1
            for key, dstT, ssb in (("q", qTr, sn_sb), ("k", kTr, sn_sb)):
                sw = pp.tile([P, QB], F32, tag="ps", name=f"sw{key}")
                nc.tensor.matmul(sw, pswap_sb, raws[key], start=True, stop=True)
                t2 = tp.tile([P, QB], F32, tag="t2")
                nc.vector.tensor_mul(out=t2, in0=sw, in1=ssb[:, sl])
                nc.vector.tensor_add(out=dstT[:, sl], in0=t1s[key], in1=t2)

            # V^T direct: per 128-key chunk, out [128 keys, 128 (2 heads x 64)]
            for sub in range(4):
                ksl = slice(st * QB + sub * KC, st * QB + (sub + 1) * KC)
                vac = pp.tile([P, QB], F32, tag="ps", name="vac")[:, 0:KC]
                for dk in range(8):
                    nc.tensor.matmul(vac, xTr_sb_chunk(xts[dk], sub), wv_sb[:, dk, :],
                                     start=(dk == 0), stop=(dk == 7))
                ci = st * 4 + sub
                nc.scalar.copy(out=vna[0][:, ci, 0:HD], in_=vac[:, 0:HD])
                nc.scalar.copy(out=vna[1][:, ci, 0:HD], in_=vac[:, HD:P])

        def xTr_sb_chunk(xt, sub):
            # lhsT for V^T: x chunk [128 dims, 128 keys]
            return xt[:, sub * KC : (sub + 1) * KC]

        chunk_counter = [0]

        def emit_attn(qsb):
            nch = 4 * (qsb + 1)
            oA = oap.tile([P, QB], F32, tag="oA")
            oB = obp.tile([P, QB], F32, tag="oB")
            pend = []
            for c in range(nch):
                is_diag = (c // 4) == qsb
                off = (c % 4) * KC if is_diag else 0
                sp = scp.tile([P, 2, QB], F32, tag="sc")
                for h in (0, 1):
                    nc.tensor.matmul(
                        sp[:, h, off:QB],
                        kTr[h * HD : (h + 1) * HD, c * KC : (c + 1) * KC],
                        qTr[h * HD : (h + 1) * HD, qsb * QB + off : (qsb + 1) * QB],
                        start=True, stop=True,
                        tile_position=(h * HD, 0),
                    )
                pt = ptp.tile([P, 2, QB], BF16, tag="pt")
                gc = chunk_counter[0]
                chunk_counter[0] += 1
                if (not is_diag) and gc % DVE_EXP_MOD == DVE_EXP_PHASE:
                    pl = plp.tile([P, 2, QB], F32, tag="pl")
                    nc.vector._custom_dve(
                        EXP_POLY3M, out=pl[:, :, off:QB], in0=sp[:, :, off:QB],
                        s0=EB3, s1=EB2, imm2=EB1)
                    nc.vector._custom_dve(
                        ADD_SCALE_POW64, out=pt[:, :, off:QB], in0=pl[:, :, off:QB],
                        s0=EA4, s1=EB0)
                else:
                    nc.scalar.activation(
                        out=pt[:, :, off:QB], in_=sp[:, :, off:QB], func=EXP,
                        scale=64.0)
                if is_diag:
                    for h in (0, 1):
                        nc.vector.tensor_mul(
                            out=pt[:, h, off : off + KC],
                            in0=pt[:, h, off : off + KC],
                            in1=tri_sb,
                        )
                if len(pend) == 2:
                    ppt, poff, pc = pend.pop(0)
                    for h, o in ((0, oA), (1, oB)):
                        nc.tensor.matmul(
                            o[0 : HD + 1, poff:QB],
                            vna[h][:, pc, :],
                            ppt[:, h, poff:QB],
                            start=(pc == 0), stop=False,
                        )
                pend.append((pt, off, c))
            for ppt, poff, pc in pend:
                for h, o in ((0, oA), (1, oB)):
                    nc.tensor.matmul(
                        o[0 : HD + 1, poff:QB],
                        vna[h][:, pc, :],
                        ppt[:, h, poff:QB],
                        start=(pc == 0), stop=(pc == nch - 1),
                    )

            # epilogue: l-row copies on ACT, recip+muls on DVE (base-0 APs only)
            rlA = rlp.tile([1, QB], F32, tag="rlA")
            nc.scalar.copy(out=rlA, in_=oA[HD : HD + 1, :])
            rlB = rlp.tile([1, QB], F32, tag="rlB")
            nc.scalar.copy(out=rlB, in_=oB[HD : HD + 1, :])
            lbA = onp_.tile([HD, QB], F32, tag="lb")
            nc.gpsimd.partition_broadcast(lbA[0:HD, :], rlA[0:1, :])
            lbB = onp_.tile([HD, QB], F32, tag="lb")
            nc.gpsimd.partition_broadcast(lbB[0:HD, :], rlB[0:1, :])
            rlbA = onp_.tile([HD, QB], F32, tag="rlb")
            nc.vector.reciprocal_approx_fast(out=rlbA, in_=lbA)
            rlbB = onp_.tile([HD, QB], F32, tag="rlb")
            nc.vector.reciprocal_approx_fast(out=rlbB, in_=lbB)

            onT = onp_.tile([P, QB], BF16, tag="onT")
            nc.vector.tensor_mul(out=onT[0:HD, :], in0=oA[0:HD, :], in1=rlbA[0:HD, :])
            oBn = onp_.tile([HD, QB], BF16, tag="oBn")
            nc.vector.tensor_mul(out=oBn[0:HD, :], in0=oB[0:HD, :], in1=rlbB[0:HD, :])
            nc.gpsimd.dma_start(out=onT[HD : 2 * HD, :], in_=oBn[0:HD, :])

            # out-projection slice
            for jc in range(8):
                fp = (oap if jc % 2 == 0 else obp).tile(
                    [P, QB], F32, tag="oA" if jc % 2 == 0 else "oB", name="fp")
                nc.tensor.matmul(fp, wo_sb[:, jc * P : (jc + 1) * P], onT,
                                 start=True, stop=True)
                fs = fsp.tile([P, QB], F32, tag="fs")
                if jc % 2 == 0:
                    nc.scalar.copy(out=fs, in_=fp)
                else:
                    nc.vector.tensor_copy(out=fs, in_=fp)
                nc.sync.dma_start(
                    out=y[jc * P : (jc + 1) * P, qsb * QB : (qsb + 1) * QB], in_=fs)

        # lag-1 pipeline: proj(0), then [proj(i+1) ; attn(i)]..., attn(7)
        emit_proj(0)
        for it_ in range(NQB):
            if it_ + 1 < NQB:
                emit_proj(it_ + 1)
            emit_attn(it_)

    nc.compile()
    return nc


def _host_prep(x, token_positions, Wq, Wk, Wv, Wo):
    import ml_dtypes

    bf16 = ml_dtypes.bfloat16
    x = np.asarray(x, dtype=np.float32)
    pos = np.asarray(token_positions).astype(np.float32)
    Wq = np.asarray(Wq, dtype=np.float32) * np.float32(QSCALE)
    Wk = np.asarray(Wk, dtype=np.float32)
    Wv = np.asarray(Wv, dtype=np.float32)
    Wo = np.asarray(Wo, dtype=np.float32)

    xT = np.ascontiguousarray(x.reshape(S, D).T)  # [D, S]

    freqs = (1.0 / THETA ** (np.arange(0, HD, 2, dtype=np.float32) / HD)).astype(
        np.float32)
    ang = pos[:, None] * freqs[None, :]          # [S, 32]
    cosv = np.cos(ang).astype(np.float32).T      # [32, S]
    sinv = np.sin(ang).astype(np.float32).T
    C64 = np.repeat(cosv, 2, axis=0)             # [64, S]
    S64 = np.empty((HD, S), dtype=np.float32)
    S64[0::2] = -sinv
    S64[1::2] = sinv
    C = np.tile(C64, (2, 1))                     # [128, S]
    Sg = np.tile(S64, (2, 1))

    pswap = np.zeros((P, P), dtype=np.float32)
    idx = np.arange(P)
    pswap[idx ^ 1, idx] = 1.0
    tri = np.triu(np.ones((P, P), dtype=np.float32))
    cst = np.concatenate([pswap, tri], axis=1)   # [128, 256]

    def b(a):
        return np.ascontiguousarray(a).astype(bf16)

    in_maps = []
    for c in range(NCORE):
        r = slice(c * P, (c + 1) * P)
        in_maps.append({
            "xT": b(xT),
            "wq": b(Wq[r, :].T),
            "wk": b(Wk[r, :].T),
            "wv": b(Wv[r, :].T),
            "wo": b(Wo[:, r].T),
            "cs": C,
            "sn": Sg,
            "cst": b(cst),
        })
    return in_maps


LAST_EXEC_NS = None
LAST_TRACE = None


def kernel(x, token_positions, Wq, Wk, Wv, Wo):
    global LAST_EXEC_NS, LAST_TRACE
    from concourse.bass_utils import run_bass_kernel_spmd

    if "nc" not in _NC_CACHE:
        _NC_CACHE["nc"] = _build_nc()
    nc = _NC_CACHE["nc"]

    in_maps = _host_prep(x, token_positions, Wq, Wk, Wv, Wo)
    res = run_bass_kernel_spmd(nc, in_maps, core_ids=list(range(NCORE)))
    LAST_EXEC_NS = res.exec_time_ns
    LAST_TRACE = (
        res.instructions_and_trace[1]
        if res.instructions_and_trace is not None
        else None
    )

    acc = np.zeros((D, S), dtype=np.float64)
    for r in res.results:
        acc += r["y"].astype(np.float64)
    out = acc.T.astype(np.float32).reshape(1, S, D)
    return out

